# revision 4
# baseline (speedup 1.0000x reference)
"""Trainium2 Bass kernel v3 for EntropyAndMutualInformation.

v3 vs v2: collective-launch floors (~33us each) dominate, so minimize
collective count and fire order:
  * ONE AllToAll for Y (after all Y softmax), queued on gpsimd between
    AG chunk 0 and AG chunk 1.
  * X AllGather in asymmetric chunks (tiles per chunk: default [2,3,2,1]);
    X chunk 0 is processed FIRST in phase 1 so AG0 fires at ~14us, then all
    Y tiles, then the remaining X chunks.
  * rhs SBUF layout is chunk-major (c, j, kt) so DoubleRow k-pairs stay
    inside a chunk for any chunk size; rhs chunk slices are loaded with one
    DMA per source rank.
  * final ln/reduce pass fused into the last chunk's waves.
Everything else (fp8 S=2048, DoubleRow, SBUF f32 accumulators) as v2.
"""
import sys

sys.path.insert(0, "/opt/trn_rl_repo")

import numpy as np

N_TOTAL, C_DIM, N_CORES = 8192, 4096, 8
EPS = 1e-12
SCALE = 2048.0


def build_nc_v4(n_total=N_TOTAL, C=C_DIM, ncores=N_CORES, xchunks=(2, 4, 2),
                wm=4, debug=False):
    import concourse.bass as bass
    import concourse.tile as tile
    import concourse.mybir as mybir
    from concourse import bacc

    f32 = mybir.dt.float32
    bf16 = mybir.dt.bfloat16
    fp8 = mybir.dt.float8e4
    P = 128
    n_shard = n_total // ncores          # 1024
    W = C // ncores                      # 512
    row_tiles = n_shard // P             # 8
    k_tiles = n_total // P               # 64
    m_tiles = C // P                     # 32
    n_waves = m_tiles // wm              # 8
    xchunks = list(xchunks)
    assert sum(xchunks) == row_tiles
    NCH = len(xchunks)
    # chunk row offsets (in my local rows)
    r0 = [sum(xchunks[:c]) * P for c in range(NCH)]          # local row start
    ckt = [xchunks[c] * ncores for c in range(NCH)]          # k-tiles/chunk
    koff = [sum(ckt[:c]) for c in range(NCH)]                # rhs k offset

    nc = bacc.Bacc("TRN2", target_bir_lowering=False, debug=debug,
                   enable_asserts=True, num_devices=ncores)

    xy_in = nc.dram_tensor("xy", [2 * n_shard, C], f32,
                           kind="ExternalInput").ap()
    x_in = xy_in[0:n_shard, :]
    y_in = xy_in[n_shard:2 * n_shard, :]

    out_all = nc.dram_tensor("out", [53, P, 1], f32,
                             kind="ExternalOutput").ap()
    zx_out = out_all[0:row_tiles]
    dx_out = out_all[row_tiles:2 * row_tiles]
    margxr_out = out_all[2 * row_tiles:2 * row_tiles + m_tiles]
    tlogt_out = out_all[48, :, :]
    margy_out = out_all[49:53].rearrange("a p o -> o (a p)")

    agx_in = nc.dram_tensor("agx_in", [n_shard, C], fp8)
    agx_out_c = [nc.dram_tensor(f"agx_out_c{c}", [ncores * xchunks[c] * P, C],
                                fp8, addr_space="Shared") for c in range(NCH)]
    a2a_in = nc.dram_tensor("a2a_in", [ncores, n_shard, W], fp8)
    a2a_out = nc.dram_tensor("a2a_out", [ncores, n_shard, W], fp8)

    Exp = mybir.ActivationFunctionType.Exp
    Ln = mybir.ActivationFunctionType.Ln
    mult = mybir.AluOpType.mult
    add = mybir.AluOpType.add
    DR = mybir.MatmulPerfMode.DoubleRow

    rg = [list(range(ncores))]

    a2a_in_v = a2a_in[:].rearrange("j (t p) w -> t p j w", p=P)
    lhs_view_c = [agx_out_c[c][:].rearrange("(k p) c -> p k c", p=P)
                  for c in range(NCH)]

    with tile.TileContext(nc) as tc:
        with (
            tc.tile_pool(name="pin", bufs=2) as pin,
            tc.tile_pool(name="pe_", bufs=2) as pe_,
            tc.tile_pool(name="ppr", bufs=2) as ppr,
            tc.tile_pool(name="pscr", bufs=1) as pscr,
            tc.tile_pool(name="p1s", bufs=8) as p1s,
            tc.tile_pool(name="rhsp", bufs=1) as rhsp,
            tc.tile_pool(name="constp", bufs=1) as constp,
            tc.tile_pool(name="slabp", bufs=2) as slabp,
            tc.tile_pool(name="jpsum", bufs=8, space="PSUM") as jpsum,
            tc.tile_pool(name="accs", bufs=1) as accs,
            tc.tile_pool(name="drainp", bufs=2) as drainp,
            tc.tile_pool(name="smallp", bufs=4) as smallp,
        ):
            ones3 = constp.tile([P, 1], fp8)
            nc.vector.memset(ones3[:], 1.0)
            ln_bias = constp.tile([P, 1], f32)
            nc.vector.memset(ln_bias[:], float(SCALE) * SCALE * n_total * EPS)

            rhs = rhsp.tile([P, k_tiles, W], fp8)
            acc = [accs.tile([P, W], f32, name=f"acc_{gm}")
                   for gm in range(m_tiles)]

            def p1_x_tile(t):
                xt = pin.tile([P, C], f32, tag="xt")
                nc.scalar.dma_start(xt[:], x_in[t * P:(t + 1) * P, :])
                ex = pe_.tile([P, C], f32, tag="et")
                zx = p1s.tile([P, 1], f32, tag="z")
                nc.scalar.activation(ex[:], xt[:], Exp, accum_out=zx[:])
                nc.scalar.dma_start(zx_out[t], zx[:])
                rzx = p1s.tile([P, 1], f32, tag="rz")
                nc.vector.reciprocal(rzx[:], zx[:])
                rzxs = p1s.tile([P, 1], f32, tag="rzs")
                nc.vector.tensor_scalar_mul(rzxs[:], rzx[:], float(SCALE))
                pxt = ppr.tile([P, C], fp8, tag="pt")
                nc.vector.tensor_scalar_mul(pxt[:], ex[:], rzxs[:])
                nc.sync.dma_start(agx_in[t * P:(t + 1) * P, :], pxt[:])
                scr = pscr.tile([P, C], bf16, tag="scr")
                dx = p1s.tile([P, 1], f32, tag="dx")
                nc.vector.scalar_tensor_tensor(
                    out=scr[:], in0=ex[:], scalar=1.0, in1=xt[:],
                    op0=mult, op1=mult, accum_out=dx[:])
                nc.sync.dma_start(dx_out[t], dx[:])

            def p1_y_tile(t):
                yt = pin.tile([P, C], f32, tag="xt")
                nc.scalar.dma_start(yt[:], y_in[t * P:(t + 1) * P, :])
                ey = pe_.tile([P, C], f32, tag="et")
                zy = p1s.tile([P, 1], f32, tag="z")
                nc.scalar.activation(ey[:], yt[:], Exp, accum_out=zy[:])
                rzy = p1s.tile([P, 1], f32, tag="rz")
                nc.vector.reciprocal(rzy[:], zy[:])
                rzys = p1s.tile([P, 1], f32, tag="rzs")
                nc.vector.tensor_scalar_mul(rzys[:], rzy[:], float(SCALE))
                pyt = ppr.tile([P, C], fp8, tag="pt")
                nc.vector.tensor_scalar_mul(pyt[:], ey[:], rzys[:])
                nc.sync.dma_start(
                    a2a_in_v[t], pyt[:].rearrange("p (j w) -> p j w", j=ncores))

            def fire_ag(c):
                nc.gpsimd.collective_compute(
                    "AllGather", mybir.AluOpType.bypass, replica_groups=rg,
                    ins=[agx_in[r0[c]:r0[c] + xchunks[c] * P, :]],
                    outs=[agx_out_c[c][:]])

            def load_rhs_chunk(c):
                # chunk-major layout: rhs[:, koff[c] + j*xt + tt, :] =
                #   qY rows of rank j, my local tile (xchunks[c] tiles per j)
                xt_ = xchunks[c]
                for j in range(ncores):
                    src = a2a_out[j, r0[c]:r0[c] + xt_ * P, :]
                    nc.sync.dma_start(
                        rhs[:, koff[c] + j * xt_:koff[c] + (j + 1) * xt_, :],
                        src.rearrange("(t p) w -> p t w", p=P))

            def mm_chunk(c, final=False):
                kc = ckt[c]
                acc_t_prev = [None]
                for w in range(n_waves):
                    psums = [jpsum.tile([P, W], f32, tag="jp",
                                        name=f"jp_{c}_{w}_{m}")
                             for m in range(wm)]
                    for ks in range(0, kc, 16):
                        ke = min(kc, ks + 16)
                        slab = slabp.tile([P, ke - ks, wm * P], fp8,
                                          tag="slab", name=f"slab_{c}_{w}_{ks}")
                        nc.sync.dma_start(
                            slab[:],
                            lhs_view_c[c][:, ks:ke,
                                          w * wm * P:(w + 1) * wm * P])
                        for kk in range(ks, ke, 2):
                            for m in range(wm):
                                nc.tensor.matmul(
                                    psums[m][:],
                                    slab[:, kk - ks:kk - ks + 2,
                                         m * P:(m + 1) * P],
                                    rhs[:, koff[c] + kk:koff[c] + kk + 2, :],
                                    start=(kk == 0), stop=(kk == kc - 2),
                                    perf_mode=DR)
                    for m in range(wm):
                        gm = w * wm + m
                        if c == 0:
                            nc.vector.tensor_copy(acc[gm][:], psums[m][:])
                        else:
                            nc.vector.tensor_add(acc[gm][:], acc[gm][:],
                                                 psums[m][:])
                        if final:
                            finalize_tile(gm, acc_t_prev)
                return acc_t_prev[0]

            def finalize_tile(gm, acc_t_prev):
                lnt = drainp.tile([P, W], bf16, tag="lnt")
                nc.scalar.activation(lnt[:], acc[gm][:], Ln, bias=ln_bias[:])
                scr1 = drainp.tile([P, W], bf16, tag="scr1")
                tt_tmp = smallp.tile([P, 1], f32, tag="tttmp")
                nc.vector.scalar_tensor_tensor(
                    out=scr1[:], in0=acc[gm][:], scalar=1.0,
                    in1=lnt[:], op0=mult, op1=mult, accum_out=tt_tmp[:])
                acc_t = smallp.tile([P, 1], f32, tag="acct",
                                    name=f"acct_{gm}")
                if acc_t_prev[0] is None:
                    nc.vector.tensor_copy(acc_t[:], tt_tmp[:])
                else:
                    nc.vector.tensor_add(acc_t[:], acc_t_prev[0][:],
                                         tt_tmp[:])
                acc_t_prev[0] = acc_t
                scr2 = drainp.tile([P, W], bf16, tag="scr2")
                st_m = smallp.tile([P, 1], f32, tag="stm")
                nc.vector.tensor_scalar(
                    out=scr2[:], in0=acc[gm][:], scalar1=1.0,
                    scalar2=None, op0=mult, op1=add, accum_out=st_m[:])
                nc.scalar.dma_start(margxr_out[gm], st_m[:])

            # ---------------- pipeline ----------------
            # X chunk 0 first -> AG0 fires early
            for tl in range(xchunks[0]):
                p1_x_tile(tl)
            fire_ag(0)
            # all Y -> single A2A (queues on gpsimd behind AG0)
            for t in range(row_tiles):
                p1_y_tile(t)
            nc.gpsimd.collective_compute(
                "AllToAll", mybir.AluOpType.bypass, replica_groups=rg,
                ins=[a2a_in[:]], outs=[a2a_out[:]])
            # remaining X chunks
            tdone = xchunks[0]
            for c in range(1, NCH):
                for tl in range(xchunks[c]):
                    p1_x_tile(tdone + tl)
                tdone += xchunks[c]
                fire_ag(c)

            # rhs (all chunks; a2a complete by now) + margy
            for c in range(NCH):
                load_rhs_chunk(c)
            psum_my = jpsum.tile([1, W], f32, tag="jp", name="psum_my")
            for k in range(k_tiles):
                nc.tensor.matmul(psum_my[:], ones3[:, 0:1], rhs[:, k, :],
                                 start=(k == 0), stop=(k == k_tiles - 1))
            margy_sb = constp.tile([1, W], f32)
            nc.vector.tensor_copy(margy_sb[:], psum_my[:])
            nc.scalar.dma_start(margy_out[:], margy_sb[:])

            # joint matmul chunks; final pass fused into last chunk
            for c in range(NCH - 1):
                mm_chunk(c)
            acc_t_last = mm_chunk(NCH - 1, final=True)
            nc.scalar.dma_start(tlogt_out[:], acc_t_last[:])

    nc.compile()
    return nc


_CACHE = {}


def _get_compiled(key=(N_TOTAL, C_DIM, N_CORES)):
    if key not in _CACHE:
        _CACHE[key] = build_nc_v4(*key)
    return _CACHE[key]


def combine_host(results, n_total=N_TOTAL, C=C_DIM, ncores=N_CORES):
    """Combine per-core partial outputs into the [2] f32 result (fp64 math)."""
    n = float(n_total)
    s2 = SCALE * SCALE
    ent_sum = 0.0
    s_tln = 0.0
    s_t = 0.0
    margx = np.zeros(C, dtype=np.float64)
    margy_blocks = []
    for r in results:
        o = r["out"].astype(np.float64).reshape(53, 128)
        z = o[0:8].ravel()
        d = o[8:16].ravel()
        ent_sum += np.sum(np.log(z) - d / z)
        s_tln += float(np.sum(o[48]))
        mxr = o[16:48].reshape(-1)
        s_t += float(mxr.sum())
        margx += mxr
        margy_blocks.append(o[49:53].ravel())
    margy = np.concatenate(margy_blocks)
    entropy = ent_sum / n
    S1 = ((s_tln - np.log(s2) * s_t) / s2 - np.log(n) * (s_t / s2)) / n
    mX = margx / (n * s2)
    mY = margy / (n * SCALE)
    mi = S1 - np.sum(mX * np.log(mX + EPS)) - np.sum(mY * np.log(mY + EPS))
    return np.array([entropy, mi], dtype=np.float32)


def kernel(act_X, act_Y):
    from concourse.bass_utils import run_bass_kernel_spmd

    act_X = np.ascontiguousarray(np.asarray(act_X, dtype=np.float32))
    act_Y = np.ascontiguousarray(np.asarray(act_Y, dtype=np.float32))
    assert act_X.shape == (N_TOTAL, C_DIM) and act_Y.shape == (N_TOTAL, C_DIM)

    nc = _get_compiled()
    n_shard = N_TOTAL // N_CORES
    in_maps = [
        {"xy": np.concatenate([act_X[k * n_shard:(k + 1) * n_shard],
                               act_Y[k * n_shard:(k + 1) * n_shard]], axis=0)}
        for k in range(N_CORES)
    ]
    res = run_bass_kernel_spmd(nc, in_maps, list(range(N_CORES)))
    return combine_host(res.results)
